# revision 8
# baseline (speedup 1.0000x reference)
"""N-ary TreeLSTM (gnn_message_passing) on 8 TRN2 NeuronCores.

Strategy: data-parallel over batch B=8, one example per core.

Key observations exploited:
  * Only the first H columns of the 3H iou_hr/iou_hl matmuls are ever used
    (the scatter touches only the i-part); the o/u parts of iou come purely
    from the loop-invariant iou_x, so o = sigmoid(iou_x[:,H:2H]) and
    u = tanh(iou_x[:,2H:3H]) are precomputed once.
  * W_fh0+W_fh1 and W_fh2+W_fh3 fold (same gather index) - folded on device.
  * All row gathers / scatter-adds are per-example [128]->[128] index maps,
    expressed as 128x128 0/1 matrices (host-built from the int tree_ids) and
    executed as TensorEngine matmuls (scatter-add duplicates handled natively).
  * torch masked_scatter_ flattens over the whole batch, so example b can pull
    rows from the tail of example b-1's h_full/c_full. Host analysis shows the
    lookback is small; each step the cores AllGather the last T rows of
    h_full/c_full, and the blend h_new = P1@h_full + Dk@h_old + P2@stack
    reproduces the exact semantics.
  * Biases enter only through per-row multiplicity counts (scatter) or
    constant rows (gather); folded via K=1 outer-product matmuls.

TensorEngine operands are bf16 (fp32 PSUM accumulate); gates/elementwise run
in fp32.  Measured end-to-end error vs the fp32 reference is ~4e-3 relative.

PSUM budget (8 banks, one [128,512]f32 tile per bank) is allocated by tag:
  y0..y3 (the four h@W products; reused by the preamble groups),
  ps_i, ps_f, ps_c, ps_b.  PE transposes reuse the ps_i/ps_f slots.
"""

import numpy as np
import ml_dtypes

BF16 = ml_dtypes.bfloat16
B, S, H, E, V, NSTEPS = 8, 128, 512, 512, 32000, 8
KT = H // 128  # contraction tiles for K=512

_last_run = None


def _one_hot_rows(idx):
    """M[j, s] = 1 iff idx[j] == s  (lhsT for scatter-add A^T @ vals)."""
    m = np.zeros((S, S), np.float32)
    m[np.arange(S), idx] = 1.0
    return m


def _host_prep(inputs):
    """Build all per-core host data derived from the integer index tensors."""
    tree = np.asarray(inputs["tree_ids"])  # [B, NSTEPS, 3, S]
    input_ids = np.asarray(inputs["input_ids"])  # [B, S]
    emb = np.asarray(inputs["emb"], dtype=np.float32)

    # ---- masked_scatter routing analysis (exact torch flat-cumsum semantics)
    # r(b,s) = number of mask-true rows strictly before flat position (b,s).
    T = 16
    per_step = []
    for t in range(NSTEPS):
        idx_d = tree[:, t, 0, :]
        mask = idx_d != 0
        flat = mask.reshape(-1)
        r_src = (np.cumsum(flat) - flat).reshape(B, S)
        for b in range(B):
            tr = np.nonzero(mask[b])[0]
            if tr.size:
                lb = int(np.max(b * S - r_src[b, tr]))
                while lb > T:
                    T *= 2
        per_step.append((idx_d, tree[:, t, 1, :], tree[:, t, 2, :], mask, r_src))
    assert T <= S, "masked_scatter lookback exceeds one example; unsupported"
    n_stack = B * T
    n_chunk = (n_stack + 127) // 128

    need_comm = [False] * NSTEPS

    core_mats = [[] for _ in range(B)]  # per core/step: [8+n_chunk,128,128]
    core_cnts = [[] for _ in range(B)]  # per core/step: [1,256]
    for t in range(NSTEPS):
        idx_d, idx_r, idx_l, mask, r_src = per_step[t]
        for b in range(B):
            Ar = _one_hot_rows(idx_r[b])
            Al = _one_hot_rows(idx_l[b])
            Ad = _one_hot_rows(idx_d[b])
            GrT = np.ascontiguousarray(Ar.T)
            GlT = np.ascontiguousarray(Al.T)
            GdT = np.ascontiguousarray(Ad.T)
            cnt_r = Ar.sum(axis=0, dtype=np.float32)
            cnt_l = Al.sum(axis=0, dtype=np.float32)
            P1 = np.zeros((S, S), np.float32)
            Dk = np.diag((~mask[b]).astype(np.float32)).astype(np.float32)
            P2c = np.zeros((n_chunk, 128, S), np.float32)
            for s in range(S):
                if not mask[b, s]:
                    continue
                src = int(r_src[b, s])
                if src >= b * S:
                    P1[src - b * S, s] = 1.0
                else:
                    q = src - ((b - 1) * S + (S - T))
                    assert 0 <= q < T, (b, s, src, T)
                    row = T * (b - 1) + q
                    P2c[row // 128, row % 128, s] = 1.0
                    need_comm[t] = True
            mats = np.stack([Ar, Al, Ad, GrT, GlT, GdT, P1, Dk], 0)
            core_mats[b].append(np.ascontiguousarray(
                np.concatenate([mats, P2c], 0)).astype(BF16))
            core_cnts[b].append(
                np.concatenate([cnt_r, cnt_l]).reshape(1, 256).astype(BF16))

    x_rows = emb[input_ids]  # [B, S, E] host gather = per-core input sharding
    return T, n_chunk, need_comm, core_mats, core_cnts, x_rows


def _build_program(T, n_chunk, need_comm):
    import concourse.bacc as bacc
    import concourse.tile as tile
    import concourse.mybir as mybir
    from contextlib import ExitStack

    dt = mybir.dt
    f32 = dt.float32
    bf16 = dt.bfloat16
    AF = mybir.ActivationFunctionType
    n_mats = 8 + n_chunk

    nc = bacc.Bacc("TRN2", target_bir_lowering=False, debug=False,
                   enable_asserts=False, num_devices=B)

    # ---------------- I/O ----------------
    x_in = nc.dram_tensor("x", [S, E], f32, kind="ExternalInput")
    w_names = ["Wr1", "Wl1", "Wfh0", "Wfh1", "Wfh2", "Wfh3", "Wfx"]
    w_ins = {n: nc.dram_tensor(n, [H, H], f32, kind="ExternalInput")
             for n in w_names}
    wioux_in = nc.dram_tensor("Wioux", [E, 3 * H], f32, kind="ExternalInput")
    bias_in = nc.dram_tensor("bias6", [6, H], f32, kind="ExternalInput")
    ident_in = nc.dram_tensor("ident", [128, 128], bf16, kind="ExternalInput")
    mats_in = [nc.dram_tensor(f"mats{t}", [n_mats, 128, 128], bf16,
                              kind="ExternalInput") for t in range(NSTEPS)]
    cnts_in = [nc.dram_tensor(f"cnts{t}", [1, 256], bf16,
                              kind="ExternalInput") for t in range(NSTEPS)]
    out_h = nc.dram_tensor("out_h", [S, H], f32, kind="ExternalOutput")

    with tile.TileContext(nc) as tc:
        with ExitStack() as ctx:
            cpool = ctx.enter_context(tc.tile_pool(name="consts", bufs=1))
            ppool = ctx.enter_context(
                tc.tile_pool(name="psum", bufs=1, space="PSUM"))
            wpool = ctx.enter_context(tc.tile_pool(name="work", bufs=2))
            mpool = ctx.enter_context(tc.tile_pool(name="mats", bufs=2))
            spool = ctx.enter_context(tc.tile_pool(name="state", bufs=2))
            dpool = ctx.enter_context(
                tc.tile_pool(name="dram", bufs=2, space="DRAM"))

            def psum(tag):
                return ppool.tile([S, H], f32, name=tag, tag=tag)

            def psumT(tag):
                # [128,128] bf16 transpose target inside a reused f32 bank
                return ppool.tile([128, 128], bf16, name="pt_" + tag, tag=tag)

            # ---------------- constants / weights ----------------
            ident = cpool.tile([128, 128], bf16, name="ident", tag="ident")
            nc.sync.dma_start(out=ident, in_=ident_in[:, :])

            # stage fp32 weights, convert to bf16 [128, KT*H] (k-tile major)
            w_sb = {}
            stage = cpool.tile([128, KT * H], f32, name="stage", tag="stage")
            stage2 = cpool.tile([128, KT * H], f32, name="stage2", tag="stage2")
            for n in ["Wr1", "Wl1", "Wfx"]:
                for k in range(KT):
                    nc.sync.dma_start(
                        out=stage[:, k * H:(k + 1) * H],
                        in_=w_ins[n][k * 128:(k + 1) * 128, :])
                w = cpool.tile([128, KT * H], bf16, name=f"w_{n}", tag=f"w_{n}")
                nc.vector.tensor_copy(w, stage)
                w_sb[n] = w
            for a, bname, oname in (("Wfh0", "Wfh1", "Wfh01"),
                                    ("Wfh2", "Wfh3", "Wfh23")):
                for k in range(KT):
                    nc.sync.dma_start(
                        out=stage[:, k * H:(k + 1) * H],
                        in_=w_ins[a][k * 128:(k + 1) * 128, :])
                    nc.sync.dma_start(
                        out=stage2[:, k * H:(k + 1) * H],
                        in_=w_ins[bname][k * 128:(k + 1) * 128, :])
                w = cpool.tile([128, KT * H], bf16, name=f"w_{oname}",
                               tag=f"w_{oname}")
                nc.vector.tensor_add(w, stage, stage2)  # fold + bf16 convert
                w_sb[oname] = w

            wioux = cpool.tile([128, KT * 3 * H], bf16, name="wioux",
                               tag="wioux")
            stage3 = cpool.tile([128, 3 * H], f32, name="stage3", tag="stage3")
            for k in range(KT):
                nc.sync.dma_start(out=stage3,
                                  in_=wioux_in[k * 128:(k + 1) * 128, :])
                nc.vector.tensor_copy(
                    wioux[:, k * 3 * H:(k + 1) * 3 * H], stage3)

            bias6 = cpool.tile([1, 6 * H], f32, name="bias6", tag="bias6")
            nc.sync.dma_start(
                out=bias6, in_=bias_in[:, :].rearrange("a c -> (a c)"))
            b_r1 = cpool.tile([1, H], bf16, name="b_r1", tag="b_r1")
            nc.vector.tensor_copy(b_r1, bias6[:, 0:H])
            b_l1 = cpool.tile([1, H], bf16, name="b_l1", tag="b_l1")
            nc.vector.tensor_copy(b_l1, bias6[:, H:2 * H])
            bf4f = cpool.tile([1, H], f32, name="bf4f", tag="bf4f")
            nc.vector.tensor_add(bf4f, bias6[:, 2 * H:3 * H],
                                 bias6[:, 3 * H:4 * H])
            nc.vector.tensor_add(bf4f, bf4f, bias6[:, 4 * H:5 * H])
            bf4 = cpool.tile([1, H], bf16, name="bf4", tag="bf4")
            nc.vector.tensor_add(bf4, bf4f, bias6[:, 5 * H:6 * H])
            ones_row = cpool.tile([1, 128], bf16, name="ones", tag="ones")
            nc.vector.memset(ones_row, 1.0)

            # ---------------- x, x^T, loop-invariants ----------------
            x_f32 = cpool.tile([S, E], f32, name="x_f32", tag="x_f32")
            nc.sync.dma_start(out=x_f32, in_=x_in[:, :])
            x_bf = cpool.tile([S, E], bf16, name="x_bf", tag="x_bf")
            nc.vector.tensor_copy(x_bf, x_f32)
            xT = cpool.tile([128, KT * 128], bf16, name="xT", tag="xT")
            for k in range(KT):
                pt = psumT("ps_i" if k % 2 == 0 else "ps_f")
                nc.tensor.transpose(pt, x_bf[:, k * 128:(k + 1) * 128], ident)
                nc.vector.tensor_copy(xT[:, k * 128:(k + 1) * 128], pt)

            # iou_x slices: iou1 (kept), o = sigmoid(slice1), u = tanh(slice2)
            iou1 = cpool.tile([S, H], bf16, name="iou1", tag="iou1")
            o_sb = cpool.tile([S, H], f32, name="o_sb", tag="o_sb")
            u_sb = cpool.tile([S, H], f32, name="u_sb", tag="u_sb")
            for i, (dest, func) in enumerate(((iou1, None), (o_sb, AF.Sigmoid),
                                              (u_sb, AF.Tanh))):
                ps = psum(f"y{i}")
                for k in range(KT):
                    nc.tensor.matmul(
                        ps,
                        xT[:, k * 128:(k + 1) * 128],
                        wioux[:, k * 3 * H + i * H:k * 3 * H + (i + 1) * H],
                        start=(k == 0), stop=(k == KT - 1))
                if func is None:
                    nc.vector.tensor_copy(dest, ps)
                else:
                    nc.scalar.activation(dest, ps, func)

            # fxb = x @ W_fx + ones x b_f4
            fxb = cpool.tile([S, H], bf16, name="fxb", tag="fxb")
            ps_fx = psum("y3")
            for k in range(KT):
                nc.tensor.matmul(ps_fx,
                                 xT[:, k * 128:(k + 1) * 128],
                                 w_sb["Wfx"][:, k * H:(k + 1) * H],
                                 start=(k == 0), stop=False)
            nc.tensor.matmul(ps_fx, ones_row, bf4, start=False, stop=True)
            nc.vector.tensor_copy(fxb, ps_fx)

            # ---------------- recurrent steps ----------------
            h_nat = hT = c_f32 = c_bf = None
            for t in range(NSTEPS):
                first = (t == 0)
                last = (t == NSTEPS - 1)

                mats = mpool.tile([128, n_mats * 128], bf16, name="mats",
                                  tag="mats")
                nc.sync.dma_start(
                    out=mats, in_=mats_in[t].ap().rearrange("m p c -> p m c"))
                cnts = mpool.tile([1, 256], bf16, name="cnts", tag="cnts")
                nc.sync.dma_start(out=cnts, in_=cnts_in[t][:, :])

                def M(i):
                    return mats[:, i * 128:(i + 1) * 128]
                Ar, Al, Ad, GrT, GlT, GdT, P1, Dk = (M(i) for i in range(8))

                y_sb = {}
                if not first:
                    for i, n in enumerate(("Wr1", "Wl1", "Wfh01", "Wfh23")):
                        ps = psum(f"y{i}")
                        for k in range(KT):
                            nc.tensor.matmul(
                                ps, hT[:, k * 128:(k + 1) * 128],
                                w_sb[n][:, k * H:(k + 1) * H],
                                start=(k == 0), stop=(k == KT - 1))
                        ysb = wpool.tile([S, H], bf16, name=f"y_{n}",
                                         tag=f"y_{n}")
                        nc.vector.tensor_copy(ysb, ps)
                        y_sb[n] = ysb

                # i-gate pre-activation: scatter-adds + count-bias + iou1
                ps_i = psum("ps_i")
                terms = [(cnts[:, 0:128], b_r1),
                         (cnts[:, 128:256], b_l1),
                         (ident, iou1)]
                if not first:
                    terms += [(Ar, y_sb["Wr1"]), (Al, y_sb["Wl1"])]
                for i, (l, r) in enumerate(terms):
                    nc.tensor.matmul(ps_i, l, r, start=(i == 0),
                                     stop=(i == len(terms) - 1))
                i_sb = wpool.tile([S, H], f32, name="i_sb", tag="i_sb")
                nc.scalar.activation(i_sb, ps_i, AF.Sigmoid)

                # f-gate: gathers of fxb / fh_r / fh_l
                ps_f = psum("ps_f")
                terms = [(GdT, fxb)]
                if not first:
                    terms += [(GrT, y_sb["Wfh01"]), (GlT, y_sb["Wfh23"])]
                for i, (l, r) in enumerate(terms):
                    nc.tensor.matmul(ps_f, l, r, start=(i == 0),
                                     stop=(i == len(terms) - 1))
                f_sb = wpool.tile([S, H], f32, name="f_sb", tag="f_sb")
                nc.scalar.activation(f_sb, ps_f, AF.Sigmoid)

                # c_full = i*u + scatter_d(f*c)
                iu = wpool.tile([S, H], bf16, name="iu", tag="iu")
                nc.vector.tensor_mul(iu, i_sb, u_sb)
                ps_c = psum("ps_c")
                nc.tensor.matmul(ps_c, ident, iu, start=True, stop=first)
                if not first:
                    fc = wpool.tile([S, H], bf16, name="fc", tag="fc")
                    nc.vector.tensor_mul(fc, f_sb, c_f32)
                    nc.tensor.matmul(ps_c, Ad, fc, start=False, stop=True)
                c_full = wpool.tile([S, H], bf16, name="c_full", tag="c_full")
                nc.vector.tensor_copy(c_full, ps_c)
                tanh_c = wpool.tile([S, H], f32, name="tanh_c", tag="tanh_c")
                nc.scalar.activation(tanh_c, ps_c, AF.Tanh)
                h_full = wpool.tile([S, H], bf16, name="h_full", tag="h_full")
                nc.vector.tensor_mul(h_full, o_sb, tanh_c)

                # ---- cross-core tail exchange (exact masked_scatter routing)
                stack = None
                if need_comm[t]:
                    ag_in = dpool.tile([T, 2 * H], bf16, name="ag_in",
                                       tag="ag_in")
                    nc.sync.dma_start(out=ag_in[:, 0:H],
                                      in_=h_full[S - T:S, :])
                    nc.sync.dma_start(out=ag_in[:, H:2 * H],
                                      in_=c_full[S - T:S, :])
                    ag_out = dpool.tile([B * T, 2 * H], bf16, name="ag_out",
                                        tag="ag_out")
                    nc.gpsimd.collective_compute(
                        "AllGather",
                        mybir.AluOpType.bypass,
                        replica_groups=[list(range(B))],
                        ins=[ag_in.opt()],
                        outs=[ag_out.opt()],
                    )
                    stack = []
                    for cc in range(n_chunk):
                        rows = min(128, B * T - cc * 128)
                        st = mpool.tile([rows, 2 * H], bf16,
                                        name=f"stack{cc}", tag=f"stack{cc}")
                        nc.sync.dma_start(
                            out=st, in_=ag_out[cc * 128:cc * 128 + rows, :])
                        stack.append(st)

                # ---- blend (masked_scatter): new = P1@full + Dk@old + P2@stk
                def blend(full, old, col, tag):
                    terms = [(P1, full)]
                    if not first:
                        terms.append((Dk, old))
                    if stack is not None:
                        for cc in range(n_chunk):
                            terms.append(
                                (M(8 + cc)[0:stack[cc].shape[0], :],
                                 stack[cc][:, col * H:(col + 1) * H]))
                    ps = psum(tag)
                    for i, (l, r) in enumerate(terms):
                        nc.tensor.matmul(ps, l, r, start=(i == 0),
                                         stop=(i == len(terms) - 1))
                    return ps

                ps_hb = blend(h_full, h_nat, 0, "ps_b")
                if last:
                    h_fin = spool.tile([S, H], f32, name="h_fin", tag="h_fin")
                    nc.vector.tensor_copy(h_fin, ps_hb)
                    nc.sync.dma_start(out=out_h[:, :], in_=h_fin)
                else:
                    h_new = spool.tile([S, H], bf16, name="h_state",
                                       tag="h_state")
                    nc.vector.tensor_copy(h_new, ps_hb)
                    ps_cb = blend(c_full, c_bf, 1, "ps_c")
                    c_f32n = spool.tile([S, H], f32, name="c_f32",
                                        tag="c_f32")
                    nc.vector.tensor_copy(c_f32n, ps_cb)
                    c_bfn = spool.tile([S, H], bf16, name="c_bf", tag="c_bf")
                    nc.vector.tensor_copy(c_bfn, ps_cb)
                    hT_new = spool.tile([128, KT * 128], bf16, name="hT_state",
                                        tag="hT_state")
                    for k in range(KT):
                        pt = psumT("ps_i" if k % 2 == 0 else "ps_f")
                        nc.tensor.transpose(
                            pt, h_new[:, k * 128:(k + 1) * 128], ident)
                        nc.vector.tensor_copy(
                            hT_new[:, k * 128:(k + 1) * 128], pt)
                    h_nat, c_f32, c_bf, hT = h_new, c_f32n, c_bfn, hT_new

    nc.compile()
    return nc


def kernel(**inputs):
    T, n_chunk, need_comm, core_mats, core_cnts, x_rows = _host_prep(inputs)

    nc = _build_program(T, n_chunk, need_comm)

    w = {k: np.ascontiguousarray(np.asarray(inputs[k], np.float32))
         for k in ("W_ioux", "W_iouh_r", "W_iouh_l", "W_fx",
                   "W_fh0", "W_fh1", "W_fh2", "W_fh3")}
    bias6 = np.stack([
        np.asarray(inputs["b_iouh_r"], np.float32)[:H],
        np.asarray(inputs["b_iouh_l"], np.float32)[:H],
        np.asarray(inputs["b_fh0"], np.float32),
        np.asarray(inputs["b_fh1"], np.float32),
        np.asarray(inputs["b_fh2"], np.float32),
        np.asarray(inputs["b_fh3"], np.float32),
    ], 0)
    ident = np.eye(128, dtype=BF16)

    shared = {
        "Wr1": np.ascontiguousarray(w["W_iouh_r"][:, :H]),
        "Wl1": np.ascontiguousarray(w["W_iouh_l"][:, :H]),
        "Wfh0": w["W_fh0"], "Wfh1": w["W_fh1"],
        "Wfh2": w["W_fh2"], "Wfh3": w["W_fh3"],
        "Wfx": w["W_fx"], "Wioux": w["W_ioux"],
        "bias6": np.ascontiguousarray(bias6),
        "ident": ident,
    }

    in_maps = []
    for b in range(B):
        m = dict(shared)
        m["x"] = np.ascontiguousarray(x_rows[b].astype(np.float32))
        for t in range(NSTEPS):
            m[f"mats{t}"] = core_mats[b][t]
            m[f"cnts{t}"] = core_cnts[b][t]
        in_maps.append(m)

    from concourse.bass_utils import run_bass_kernel_spmd
    res = run_bass_kernel_spmd(nc, in_maps, core_ids=list(range(B)))
    global _last_run
    _last_run = res
    out = np.stack([res.results[b]["out_h"] for b in range(B)], 0)
    return out.astype(np.float32)
